# revision 40
# baseline (speedup 1.0000x reference)
"""Trainium2 Bass kernel: MultiHeadAttention + residual + LayerNorm.

Problem shapes (hardcoded):
  x: (2, 2048, 1024) f32, 16 heads x 64 head_dim, scale = 64**-0.5
  y = LayerNorm(x + MHA(x))

Sharding: token-parallel over 8 cores. Core c handles batch b=c//4 and
query tokens [512*(c%4), 512*(c%4+1)) of that batch. Each core receives
its batch's full token sequence ROTATED so that its own 512 query tokens
are rows 0..511 (attention is permutation-invariant over keys, so K/V
token order does not matter). No cross-core collectives needed.

Schedule: software-pipelined across head pairs. Pair p's attention
chunks are interleaved at emission time with pair p+1's K/V projection
matmuls so the PE never head-of-line blocks on ScalarE's softmax exp.
Score matmuls for the two heads of a pair are issued adjacently with
disjoint PE row groups (contract dim 64, base partitions 0 and 64) so
they execute concurrently in the systolic array. All matmul operands
are bf16 (host-cast); V^T -> V transposes ride the DMA xbar instead of
the PE.
"""

import sys

sys.path.insert(0, "/opt/trn_rl_repo")

import numpy as np
import ml_dtypes

import concourse.bass as bass
import concourse.bacc as bacc
import concourse.mybir as mybir
import concourse.tile as tile
from concourse import bass_utils
from concourse.masks import make_identity

# ---- problem constants ----
B = 2
S = 2048
D = 1024
H = 16
DH = 64
SCALE = DH ** -0.5
EPS = 1e-5

N_CORES = 8
CORES_PER_BATCH = N_CORES // B
TQ = S // CORES_PER_BATCH          # 512 query tokens per core
NT = S // 128                      # 16 key tiles of 128
ND = D // 128                      # 8 dim tiles of 128
NPAIR = H // 2                     # 8 head pairs
NTQ = TQ // 128                    # 4 query tiles

F32 = mybir.dt.float32
BF16 = mybir.dt.bfloat16
FP8 = mybir.dt.float8e4

N_WARMUP_MM = 36                   # ~9.5us of PE warmup to lift HAM throttle


def _build_program():
    nc = bacc.Bacc("TRN2", target_bir_lowering=False, debug=False,
                   num_devices=N_CORES)

    # ---- DRAM I/O ----
    # x host-pretransposed AND host-cast to bf16: xbT[p, d, t] = x[t, 128d+p]
    xbT_d = nc.dram_tensor("xbT", (128, ND, S), FP8, kind="ExternalInput").ap()
    # xqb = x[0:TQ] + bo (residual with out-proj bias folded in), f32
    xqb_d = nc.dram_tensor("xqb", (TQ, D), F32, kind="ExternalInput").ap()
    # weights host-packed bf16: wX[p, otile, dtile, c] = WX[128*dtile+p, 128*otile+c]
    wq_d = nc.dram_tensor("wq", (128, ND, ND, 128), FP8,
                          kind="ExternalInput").ap()
    wk_d = nc.dram_tensor("wk", (128, ND, ND, 128), FP8,
                          kind="ExternalInput").ap()
    wv_d = nc.dram_tensor("wv", (128, ND, ND, 128), FP8,
                          kind="ExternalInput").ap()
    # wo[p, dtile, o] = Wo[128*dtile+p, o]
    wo_d = nc.dram_tensor("wo", (128, ND, D), FP8, kind="ExternalInput").ap()
    # biases host-packed [p, otile]
    bq_d = nc.dram_tensor("bq", (128, ND), F32, kind="ExternalInput").ap()
    bk_d = nc.dram_tensor("bk", (128, ND), F32, kind="ExternalInput").ap()
    bv_d = nc.dram_tensor("bv", (128, ND), F32, kind="ExternalInput").ap()
    gamma_d = nc.dram_tensor("gamma", (D,), F32, kind="ExternalInput").ap()
    beta_d = nc.dram_tensor("beta", (D,), F32, kind="ExternalInput").ap()
    y_d = nc.dram_tensor("y", (TQ, D), F32, kind="ExternalOutput").ap()

    def bcast_rows(src_row_ap, nrows):
        # replicate a [1, N] AP across nrows partitions (DMA only)
        return bass.AP(tensor=src_row_ap.tensor, offset=src_row_ap.offset,
                       ap=[[0, nrows]] + [list(d) for d in src_row_ap.ap[-1:]])

    with tile.TileContext(nc) as tc:
        from contextlib import ExitStack
        with ExitStack() as ctx:
            # ---- pools ----
            consts = ctx.enter_context(tc.tile_pool(name="consts", bufs=1))
            bigp = ctx.enter_context(tc.tile_pool(name="big", bufs=1))
            wpool = ctx.enter_context(tc.tile_pool(name="wpool", bufs=2))
            kvp = ctx.enter_context(tc.tile_pool(name="kvp", bufs=2))
            vtsp = ctx.enter_context(tc.tile_pool(name="vts", bufs=4))
            expp = ctx.enter_context(tc.tile_pool(name="expp", bufs=4))
            smallp = ctx.enter_context(tc.tile_pool(name="small", bufs=2))
            ybufp = ctx.enter_context(tc.tile_pool(name="ybuf", bufs=2))

            # PSUM: "sc" 4 banks x1, "pav" 1 bank x2, "acc" 1 bank x2 = 8
            ps = ctx.enter_context(tc.tile_pool(name="ps", bufs=1,
                                                space="PSUM"))

            # ---- constants / small loads (gpsimd SWDGE ring) ----
            warm = consts.tile([128, 512], BF16)
            nc.vector.memset(warm, 0.0)
            ones_r = consts.tile([128, 64], BF16)
            nc.vector.memset(ones_r, 1.0)
            ident = consts.tile([128, 128], F32)
            make_identity(nc, ident)
            ident_s = consts.tile([128, 128], BF16)
            nc.vector.tensor_copy(out=ident_s, in_=ident)
            eps_t = consts.tile([128, 1], F32)
            nc.vector.memset(eps_t, EPS)
            # first Q weight slices lead the gpsimd queue so the Q matmuls
            # are never gated on weight arrival
            wq_list = []
            for j in range(ND):
                wq_s = wpool.tile([128, ND, 128], FP8, tag="wq", bufs=8,
                                  name=f"wq_s{j}")
                nc.gpsimd.dma_start(out=wq_s, in_=wq_d[:, j, :, :])
                wq_list.append(wq_s)
            bq_t = consts.tile([128, ND], F32)
            nc.gpsimd.dma_start(out=bq_t, in_=bq_d)
            bk_t = consts.tile([128, ND], F32)
            nc.gpsimd.dma_start(out=bk_t, in_=bk_d)
            bv_t = consts.tile([128, ND], F32)
            nc.gpsimd.dma_start(out=bv_t, in_=bv_d)

            # ---- PE warmup: keep HAM at 8/8 while x streams in ----
            wps = ps.tile([128, 2, 512], F32, tag="sca", name="wps")
            for i in range(N_WARMUP_MM):
                nc.tensor.matmul(wps[:, 0, :], warm[:, 0:128], warm,
                                 start=True, stop=True)

            # ---- x load: direct bf16, split across both HWDGE rings ----
            xT = bigp.tile([128, ND, S], FP8)
            for d in range(ND):
                eng = nc.sync if d < 4 else nc.scalar
                eng.dma_start(out=xT[:, d, :], in_=xbT_d[:, d, :])

            # =========================================================
            # Q projection: qT[p, j, tq] = q[tq, 128j+p], own tokens
            # =========================================================
            qT = bigp.tile([128, ND, TQ], BF16)

            def emit_q_piece(j):
                wq_s = wq_list[j]
                pq = ps.tile([128, 512], F32, tag="acc", bufs=2, name="pq")
                # fp8 DoubleRow: each matmul contracts TWO d-tiles; order
                # follows x-DMA arrival (d0-3 on sync, d4-7 on scalar)
                dorder = [0, 4, 2, 6]
                for di, d in enumerate(dorder):
                    nc.tensor.matmul(pq, wq_s[:, d:d + 2, :],
                                     xT[:, d:d + 2, 0:TQ],
                                     start=(di == 0), stop=(di == 3),
                                     perf_mode=mybir.MatmulPerfMode.DoubleRow)
                nc.vector.tensor_scalar_add(
                    out=qT[:, j, :], in0=pq, scalar1=bq_t[:, j:j + 1])

            emit_q_piece(0)  # pair 0's queries; j=1..7 drain in the stream

            # =========================================================
            # Per-pair projection pieces (emitted interleaved, below)
            # =========================================================
            kT_s = [None] * NPAIR     # [128, 4, 512] bf16 per pair
            va_s = [None] * NPAIR     # [128, NT, 2, 66] bf16 per pair
            wk_ss = [None] * NPAIR
            wv_ss = [None] * NPAIR

            def emit_wdma(p):
                wk_ss[p] = wpool.tile([128, ND, 128], FP8, tag="wk",
                                      name=f"wk{p}")
                nc.gpsimd.dma_start(out=wk_ss[p], in_=wk_d[:, p, :, :])
                wv_ss[p] = wpool.tile([128, ND, 128], FP8, tag="wv",
                                      name=f"wv{p}")
                nc.gpsimd.dma_start(out=wv_ss[p], in_=wv_d[:, p, :, :])

            def emit_k_piece(p, n):
                if n == 0:
                    kT_s[p] = kvp.tile([128, NT // 4, 512], BF16, tag="kT",
                                       name=f"kT{p}")
                pk = ps.tile([128, 512], F32, tag="acc", bufs=2, name="pk")
                for di in range(4):
                    d = 2 * di
                    nc.tensor.matmul(pk, wk_ss[p][:, d:d + 2, :],
                                     xT[:, d:d + 2, 512 * n:512 * (n + 1)],
                                     start=(di == 0), stop=(di == 3),
                                     perf_mode=mybir.MatmulPerfMode.DoubleRow)
                nc.vector.tensor_scalar_add(
                    out=kT_s[p][:, n, :], in0=pk, scalar1=bk_t[:, p:p + 1])

            def emit_v_piece(p, n):
                if n == 0:
                    va_s[p] = kvp.tile([128, NT, 2, 72], FP8, tag="va",
                                       name=f"va{p}")
                    nc.vector.memset(va_s[p][:, :, :, 64:65], 1.0)
                pv = ps.tile([128, 512], F32, tag="acc", bufs=2, name="pv")
                for di in range(4):
                    d = 2 * di
                    nc.tensor.matmul(pv, wv_ss[p][:, d:d + 2, :],
                                     xT[:, d:d + 2, 512 * n:512 * (n + 1)],
                                     start=(di == 0), stop=(di == 3),
                                     perf_mode=mybir.MatmulPerfMode.DoubleRow)
                vts_t = vtsp.tile([128, 512], BF16, tag="vts", name="vts")
                nc.vector.tensor_scalar_add(
                    out=vts_t, in0=pv, scalar1=bv_t[:, p:p + 1])
                # PE transpose V^T -> V, 4 token blocks into one PSUM bank
                tr = ps.tile([128, 4, 128], BF16, tag="acc", bufs=2,
                             name="tr")
                for s in range(4):
                    nc.tensor.transpose(
                        tr[:, s, :], vts_t[:, 128 * s:128 * (s + 1)],
                        ident_s)
                for s in range(4):
                    t = 4 * n + s
                    nc.vector.tensor_copy(
                        out=va_s[p][:, t, :, 0:64],
                        in_=tr[:, s, :].rearrange("p (h l) -> p h l", h=2))

            # pair 0 projections up front (paced by the x DMAs)
            emit_wdma(0)
            # phase-D inputs queued on gpsimd behind the hot weight loads
            lnc = bigp.tile([128, 2, D], F32)
            nc.gpsimd.dma_start(out=lnc[:, 0, :],
                                in_=bcast_rows(gamma_d[None], 128))
            nc.gpsimd.dma_start(out=lnc[:, 1, :],
                                in_=bcast_rows(beta_d[None], 128))
            gamma_b, beta_b = lnc[:, 0, :], lnc[:, 1, :]
            lnc_bf = bigp.tile([128, 2, D], BF16)
            nc.vector.tensor_copy(out=lnc_bf, in_=lnc)
            gamma_bf, beta_bf = lnc_bf[:, 0, :], lnc_bf[:, 1, :]
            xqb_t = bigp.tile([128, NTQ, D], F32)
            nc.gpsimd.dma_start(
                out=xqb_t, in_=xqb_d.rearrange("(i p) d -> p i d", p=128))
            for n in range(4):
                emit_k_piece(0, n)

            # =========================================================
            # Attention, software-pipelined across pairs
            # =========================================================
            outT = bigp.tile([128, ND, TQ], FP8)
            wo_t = bigp.tile([128, ND, D], FP8)  # DMA'd as pair-7 work

            def emit_scores_group(p, ch, g, exs):
                # per-head score buffers: the two streams ping-pong on
                # separate PSUM tags so scores(k+1) of head A only waits on
                # ACT_a(k), never on the full slot's worth of exp work
                psc_a = ps.tile([128, 2, 512], F32, tag="sca", name="psc_a")
                psc_b = ps.tile([128, 2, 512], F32, tag="scb", name="psc_b")
                for s2 in range(2):
                    t = ch * 4 + g * 2 + s2
                    nt, tt = t // 4, t % 4
                    ksl = kT_s[p][:, nt, 128 * tt:128 * (tt + 1)]
                    # two heads adjacent, disjoint row groups -> run
                    # concurrently in the PE array
                    nc.tensor.matmul(psc_a[:, s2, :], ksl[0:64, :],
                                     qT[0:64, p, :], start=True, stop=True)
                    nc.tensor.matmul(psc_b[:, s2, :], ksl[64:128, :],
                                     qT[64:128, p, :], start=True, stop=True)
                ex_a = expp.tile([128, 2, 512], FP8, tag="exa", name="ex_a")
                nc.scalar.activation(
                    out=ex_a, in_=psc_a,
                    func=mybir.ActivationFunctionType.Exp, scale=SCALE / 256.0)
                ex_b = expp.tile([128, 2, 512], FP8, tag="exb", name="ex_b")
                nc.scalar.activation(
                    out=ex_b, in_=psc_b,
                    func=mybir.ActivationFunctionType.Exp, scale=SCALE / 256.0)
                exs[(ch, g)] = (ex_a, ex_b)

            def emit_attnv_group(p, ch, g, exs, pav):
                # fp8 DoubleRow: one matmul contracts BOTH key tiles of the
                # group (2 x 128 keys), ~1.44x PE throughput
                ex_pair = exs[(ch, g)]
                t = ch * 4 + g * 2
                for he in range(2):
                    nc.tensor.matmul(
                        pav[he][0:65, :],
                        va_s[p][:, t:t + 2, he, 0:65],
                        ex_pair[he],
                        start=(t == 0), stop=(t == NT - 2),
                        perf_mode=mybir.MatmulPerfMode.DoubleRow)

            def emit_normalize(p, pav):
                # denominator: row 64 of pav -> K=1 matmul broadcast to 64
                # partitions -> reciprocal -> multiply.
                for he in range(2):
                    dns = smallp.tile([128, TQ], BF16, tag="dns", name="dns")
                    nc.vector.tensor_copy(out=dns[64:65, :],
                                          in_=pav[he][64:65, :])
                    rb = ps.tile([128, 512], F32, tag="acc", bufs=2,
                                 name="rb")
                    nc.tensor.matmul(rb[0:64, :], ones_r[64:65, :],
                                     dns[64:65, :], start=True, stop=True)
                    scr = smallp.tile([128, TQ], F32, tag="scr", name="scr")
                    rrec = smallp.tile([128, TQ], F32, tag="rrec",
                                       name="rrec")
                    nc.vector.reciprocal_approx_accurate(
                        out=rrec[0:64, :], in_=rb[0:64, :],
                        scratch=scr[0:64, :])
                    if he == 0:
                        nc.vector.tensor_mul(
                            out=outT[0:64, p, :],
                            in0=pav[he][0:64, :], in1=rrec[0:64, :])
                    else:
                        tmp = smallp.tile([128, TQ], FP8, tag="tmp",
                                          name="tmp")
                        nc.vector.tensor_mul(
                            out=tmp[0:64, :],
                            in0=pav[he][0:64, :], in1=rrec[0:64, :])
                        nc.gpsimd.dma_start(
                            out=outT[64:128, p, :], in_=tmp[0:64, :])

            # Flat slot stream (p, ch, g): scores for slot k, attn.V for
            # slot k-1, one projection piece -- the lag-1 attn.V crosses
            # pair boundaries so the PE always has ~2us of queued work
            # while the exp ACT for the freshly issued scores completes.
            exs = {}
            pavs = {}
            work = [lambda nn=n: emit_v_piece(0, nn) for n in range(4)]
            prev = None
            norm_pending = None
            slots = [(p, ch, g) for p in range(NPAIR)
                     for ch in range(4) for g in range(2)]
            for (p, ch, g) in slots:
                if ch == 0 and g == 0:
                    pavs[p] = [ps.tile([128, 512], F32, tag="pav", bufs=2,
                                       name=f"pav{p}_{he}")
                               for he in range(2)]
                    if p + 1 < NPAIR:
                        work.append(lambda pp=p + 1: emit_q_piece(pp))
                        work.append(lambda pp=p + 1: emit_wdma(pp))
                        for n in range(4):
                            work.append(lambda pp=p + 1, nn=n:
                                        emit_k_piece(pp, nn))
                        for n in range(4):
                            work.append(lambda pp=p + 1, nn=n:
                                        emit_v_piece(pp, nn))
                    else:
                        work.append(
                            lambda: nc.gpsimd.dma_start(out=wo_t, in_=wo_d))
                emit_scores_group(p, ch, g, exs)
                if prev is not None:
                    pp, pch, pg = prev
                    emit_attnv_group(pp, pch, pg, exs, pavs[pp])
                    if (pch, pg) == (3, 1):
                        norm_pending = pp
                if work:
                    work.pop(0)()
                if ch == 1 and g == 1 and work:
                    work.pop(0)()  # second drain slot: 10 pieces per pair
                if ch == 3 and g == 1 and work:
                    work.pop(0)()  # keep up: 10 pieces arrive per pair
                if norm_pending is not None and (ch, g) == (1, 0):
                    # deferred 2 extra slots: the dns DVE copy has drained
                    # by the time the PE reaches the rb broadcast matmul
                    emit_normalize(norm_pending, pavs.pop(norm_pending))
                    norm_pending = None
                prev = (p, ch, g)
            emit_attnv_group(7, 3, 1, exs, pavs[7])
            emit_normalize(7, pavs.pop(7))
            while work:
                work.pop(0)()

            # =========================================================
            # Out-projection + residual + LayerNorm
            # =========================================================
            for i in range(NTQ):
                # two PSUM halves on different tags -> i and i+1 overlap
                po_h = [ps.tile([128, 512], F32, tag="pav", bufs=2,
                                name="poa"),
                        ps.tile([128, 512], F32, tag="acc", bufs=2,
                                name="pob")]
                for half in range(2):
                    for di in range(4):
                        d = 2 * di
                        nc.tensor.matmul(
                            po_h[half],
                            outT[:, d:d + 2, 128 * i:128 * (i + 1)],
                            wo_t[:, d:d + 2, 512 * half:512 * (half + 1)],
                            start=(di == 0), stop=(di == 3),
                            perf_mode=mybir.MatmulPerfMode.DoubleRow)
                ysb = ybufp.tile([128, D], F32, tag="ysb", name="ysb")
                # y = out + (x + bo)  (bo folded into xqb on host)
                for half in range(2):
                    nc.vector.tensor_add(
                        out=ysb[:, 512 * half:512 * (half + 1)],
                        in0=po_h[half],
                        in1=xqb_t[:, i, 512 * half:512 * (half + 1)])
                # LayerNorm
                stats = smallp.tile([128, 2, 6], F32, tag="stats")
                mv = smallp.tile([128, 2], F32, tag="mv")
                yv = ysb.rearrange("p (a b) -> p a b", a=2)
                for sg in range(2):
                    nc.vector.bn_stats(out=stats[:, sg, :], in_=yv[:, sg, :])
                nc.vector.bn_aggr(out=mv, in_=stats)
                sd = smallp.tile([128, 1], F32, tag="sd")
                nc.scalar.activation(out=sd, in_=mv[:, 1:2],
                                     func=mybir.ActivationFunctionType.Sqrt,
                                     bias=eps_t, scale=1.0)
                rstd = smallp.tile([128, 1], F32, tag="rstd")
                nc.vector.reciprocal(out=rstd, in_=sd)
                # post-stats chain entirely on DVE in bf16 (2x rate); the
                # normalized values are O(1) so bf16 rounding is ~0.4%
                ysb2 = ybufp.tile([128, D], BF16, tag="ysb2", name="ysb2")
                nc.vector.tensor_scalar(
                    out=ysb2, in0=ysb, scalar1=mv[:, 0:1],
                    scalar2=rstd, op0=mybir.AluOpType.subtract,
                    op1=mybir.AluOpType.mult)
                nc.vector.tensor_mul(out=ysb2, in0=ysb2, in1=gamma_bf)
                nc.vector.tensor_add(out=ysb, in0=ysb2, in1=beta_bf)
                nc.sync.dma_start(out=y_d[128 * i:128 * (i + 1), :],
                                  in_=ysb)

    nc.compile()
    return nc


_PROGRAM_CACHE = {}


def _get_program():
    if "p" not in _PROGRAM_CACHE:
        _PROGRAM_CACHE["p"] = _build_program()
    return _PROGRAM_CACHE["p"]


def _pack_w(w):
    # [p, otile, dtile, c] = W[128*dtile+p, 128*otile+c], x16 into fp8's
    # normal range (compensated via exp scale / ones column on device)
    w = np.asarray(w, np.float32).reshape(ND, 128, ND, 128)
    return np.ascontiguousarray(
        w.transpose(1, 2, 0, 3) * 16.0).astype(ml_dtypes.float8_e4m3fn)


def _pack_wo(w):
    # [p, dtile, o] = 16*W[128*dtile+p, o], fp8 (outT carries x16 as well;
    # the combined x256 also scales the residual and cancels in LayerNorm)
    w = np.asarray(w, np.float32).reshape(ND, 128, D)
    return np.ascontiguousarray(
        w.transpose(1, 0, 2) * 16.0).astype(ml_dtypes.float8_e4m3fn)


def _pack_b(b):
    # [p, otile] = 16*b[128*otile+p]  (matches the x16 weight scaling)
    b = np.asarray(b, np.float32).reshape(ND, 128)
    return np.ascontiguousarray(b.transpose(1, 0)) * 16.0


def kernel(x, Wq, bq, Wk, bk, Wv, bv, Wo, bo, gamma, beta, _trace=False):
    x = np.asarray(x, dtype=np.float32)
    nc = _get_program()

    wq_p, wk_p, wv_p = _pack_w(Wq), _pack_w(Wk), _pack_w(Wv)
    wo_p = _pack_wo(Wo)
    bq_p, bk_p, bv_p = _pack_b(bq), _pack_b(bk), _pack_b(bv)
    bo_f = np.asarray(bo, np.float32)
    in_maps = []
    for c in range(N_CORES):
        b = c // CORES_PER_BATCH
        off = TQ * (c % CORES_PER_BATCH)
        xb = np.concatenate([x[b, off:], x[b, :off]], axis=0)
        xbT = np.ascontiguousarray(
            xb.T.reshape(ND, 128, S).transpose(1, 0, 2)).astype(
                ml_dtypes.float8_e4m3fn)
        in_maps.append({
            "xbT": xbT,
            "xqb": (np.ascontiguousarray(xb[0:TQ]) + bo_f) * 256.0,
            "wq": wq_p, "wk": wk_p, "wv": wv_p, "wo": wo_p,
            "bq": bq_p, "bk": bk_p, "bv": bv_p,
            "gamma": np.asarray(gamma, np.float32),
            "beta": np.asarray(beta, np.float32),
        })

    res = bass_utils.run_bass_kernel_spmd(
        nc, in_maps, list(range(N_CORES)), trace=_trace)

    y = np.empty((B, S, D), dtype=np.float32)
    for c in range(N_CORES):
        b = c // CORES_PER_BATCH
        off = TQ * (c % CORES_PER_BATCH)
        y[b, off:off + TQ] = res.results[c]["y"]

    kernel.last_exec_time_ns = res.exec_time_ns
    return y


kernel.last_exec_time_ns = None


# revision 41
# speedup vs baseline: 1.2037x; 1.2037x over previous
"""Trainium2 Bass kernel: MultiHeadAttention + residual + LayerNorm.

Problem shapes (hardcoded):
  x: (2, 2048, 1024) f32, 16 heads x 64 head_dim, scale = 64**-0.5
  y = LayerNorm(x + MHA(x))

Sharding: token-parallel over 8 cores. Core c handles batch b=c//4 and
query tokens [512*(c%4), 512*(c%4+1)) of that batch. Each core receives
its batch's full token sequence ROTATED so that its own 512 query tokens
are rows 0..511 (attention is permutation-invariant over keys, so K/V
token order does not matter). No cross-core collectives needed.

Schedule: software-pipelined across head pairs. Pair p's attention
chunks are interleaved at emission time with pair p+1's K/V projection
matmuls so the PE never head-of-line blocks on ScalarE's softmax exp.
Score matmuls for the two heads of a pair are issued adjacently with
disjoint PE row groups (contract dim 64, base partitions 0 and 64) so
they execute concurrently in the systolic array. All matmul operands
are bf16 (host-cast); V^T -> V transposes ride the DMA xbar instead of
the PE.
"""

import sys

sys.path.insert(0, "/opt/trn_rl_repo")

import numpy as np
import ml_dtypes

import concourse.bass as bass
import concourse.bacc as bacc
import concourse.mybir as mybir
import concourse.tile as tile
from concourse import bass_utils
from concourse.masks import make_identity

# ---- problem constants ----
B = 2
S = 2048
D = 1024
H = 16
DH = 64
SCALE = DH ** -0.5
EPS = 1e-5

N_CORES = 8
CORES_PER_BATCH = N_CORES // B
TQ = S // CORES_PER_BATCH          # 512 query tokens per core
NT = S // 128                      # 16 key tiles of 128
ND = D // 128                      # 8 dim tiles of 128
NPAIR = H // 2                     # 8 head pairs
NTQ = TQ // 128                    # 4 query tiles

F32 = mybir.dt.float32
BF16 = mybir.dt.bfloat16
FP8 = mybir.dt.float8e4

N_WARMUP_MM = 36                   # ~9.5us of PE warmup to lift HAM throttle


def _build_program():
    nc = bacc.Bacc("TRN2", target_bir_lowering=False, debug=False,
                   num_devices=N_CORES)

    # ---- DRAM I/O ----
    # x host-pretransposed AND host-cast to bf16: xbT[p, d, t] = x[t, 128d+p]
    xbT_d = nc.dram_tensor("xbT", (128, ND, S), FP8, kind="ExternalInput").ap()
    # xqb = x[0:TQ] + bo (residual with out-proj bias folded in), f32
    xqb_d = nc.dram_tensor("xqb", (TQ, D), F32, kind="ExternalInput").ap()
    # weights host-packed bf16: wX[p, otile, dtile, c] = WX[128*dtile+p, 128*otile+c]
    wq_d = nc.dram_tensor("wq", (128, ND, ND, 128), FP8,
                          kind="ExternalInput").ap()
    wk_d = nc.dram_tensor("wk", (128, ND, ND, 128), FP8,
                          kind="ExternalInput").ap()
    wv_d = nc.dram_tensor("wv", (128, ND, ND, 128), FP8,
                          kind="ExternalInput").ap()
    # wo[p, dtile, o] = Wo[128*dtile+p, o]
    wo_d = nc.dram_tensor("wo", (128, ND, D), FP8, kind="ExternalInput").ap()
    # biases host-packed [p, otile]
    bq_d = nc.dram_tensor("bq", (128, ND), F32, kind="ExternalInput").ap()
    bk_d = nc.dram_tensor("bk", (128, ND), F32, kind="ExternalInput").ap()
    bv_d = nc.dram_tensor("bv", (128, ND), F32, kind="ExternalInput").ap()
    gamma_d = nc.dram_tensor("gamma", (D,), F32, kind="ExternalInput").ap()
    beta_d = nc.dram_tensor("beta", (D,), F32, kind="ExternalInput").ap()
    y_d = nc.dram_tensor("y", (TQ, D), F32, kind="ExternalOutput").ap()

    def bcast_rows(src_row_ap, nrows):
        # replicate a [1, N] AP across nrows partitions (DMA only)
        return bass.AP(tensor=src_row_ap.tensor, offset=src_row_ap.offset,
                       ap=[[0, nrows]] + [list(d) for d in src_row_ap.ap[-1:]])

    with tile.TileContext(nc) as tc:
        from contextlib import ExitStack
        with ExitStack() as ctx:
            # ---- pools ----
            consts = ctx.enter_context(tc.tile_pool(name="consts", bufs=1))
            bigp = ctx.enter_context(tc.tile_pool(name="big", bufs=1))
            wpool = ctx.enter_context(tc.tile_pool(name="wpool", bufs=2))
            kvp = ctx.enter_context(tc.tile_pool(name="kvp", bufs=2))
            vtsp = ctx.enter_context(tc.tile_pool(name="vts", bufs=4))
            expp = ctx.enter_context(tc.tile_pool(name="expp", bufs=4))
            smallp = ctx.enter_context(tc.tile_pool(name="small", bufs=2))
            ybufp = ctx.enter_context(tc.tile_pool(name="ybuf", bufs=2))

            # PSUM: "sc" 4 banks x1, "pav" 1 bank x2, "acc" 1 bank x2 = 8
            ps = ctx.enter_context(tc.tile_pool(name="ps", bufs=1,
                                                space="PSUM"))

            # ---- constants / small loads (gpsimd SWDGE ring) ----
            warm = consts.tile([128, 512], BF16)
            nc.vector.memset(warm, 0.0)
            ones_r = consts.tile([128, 64], BF16)
            nc.vector.memset(ones_r, 1.0)
            ident = consts.tile([128, 128], F32)
            make_identity(nc, ident)
            ident_s = consts.tile([128, 128], BF16)
            nc.vector.tensor_copy(out=ident_s, in_=ident)
            eps_t = consts.tile([128, 1], F32)
            nc.vector.memset(eps_t, EPS)
            # first Q weight slices lead the gpsimd queue so the Q matmuls
            # are never gated on weight arrival
            wq_list = []
            for j in range(ND):
                wq_s = wpool.tile([128, ND, 128], FP8, tag="wq", bufs=8,
                                  name=f"wq_s{j}")
                nc.gpsimd.dma_start(out=wq_s, in_=wq_d[:, j, :, :])
                wq_list.append(wq_s)
            bq_t = consts.tile([128, ND], F32)
            nc.gpsimd.dma_start(out=bq_t, in_=bq_d)
            bk_t = consts.tile([128, ND], F32)
            nc.gpsimd.dma_start(out=bk_t, in_=bk_d)
            bv_t = consts.tile([128, ND], F32)
            nc.gpsimd.dma_start(out=bv_t, in_=bv_d)

            # ---- PE warmup: keep HAM at 8/8 while x streams in ----
            wps = ps.tile([128, 2, 512], F32, tag="sca", name="wps")
            for i in range(N_WARMUP_MM):
                nc.tensor.matmul(wps[:, 0, :], warm[:, 0:128], warm,
                                 start=True, stop=True)

            # ---- x load: direct bf16, split across both HWDGE rings ----
            xT = bigp.tile([128, ND, S], FP8)
            for d in range(ND):
                eng = nc.sync if d < 4 else nc.scalar
                eng.dma_start(out=xT[:, d, :], in_=xbT_d[:, d, :])

            # =========================================================
            # Q projection: qT[p, j, tq] = q[tq, 128j+p], own tokens
            # =========================================================
            qT = bigp.tile([128, ND, TQ], BF16)
            for j in range(ND):
                wq_s = wq_list[j]
                pq = ps.tile([128, 512], F32, tag="acc", bufs=2, name="pq")
                # fp8 DoubleRow: each matmul contracts TWO d-tiles; order
                # follows x-DMA arrival (d0-3 on sync, d4-7 on scalar)
                dorder = [0, 4, 2, 6]
                for di, d in enumerate(dorder):
                    nc.tensor.matmul(pq, wq_s[:, d:d + 2, :],
                                     xT[:, d:d + 2, 0:TQ],
                                     start=(di == 0), stop=(di == 3),
                                     perf_mode=mybir.MatmulPerfMode.DoubleRow)
                nc.vector.tensor_scalar_add(
                    out=qT[:, j, :], in0=pq, scalar1=bq_t[:, j:j + 1])

            # =========================================================
            # Per-pair projection pieces (emitted interleaved, below)
            # =========================================================
            kT_s = [None] * NPAIR     # [128, 4, 512] bf16 per pair
            va_s = [None] * NPAIR     # [128, NT, 2, 66] bf16 per pair
            wk_ss = [None] * NPAIR
            wv_ss = [None] * NPAIR

            def emit_wdma(p):
                wk_ss[p] = wpool.tile([128, ND, 128], FP8, tag="wk",
                                      name=f"wk{p}")
                nc.gpsimd.dma_start(out=wk_ss[p], in_=wk_d[:, p, :, :])
                wv_ss[p] = wpool.tile([128, ND, 128], FP8, tag="wv",
                                      name=f"wv{p}")
                nc.gpsimd.dma_start(out=wv_ss[p], in_=wv_d[:, p, :, :])

            def emit_k_piece(p, n):
                if n == 0:
                    kT_s[p] = kvp.tile([128, NT // 4, 512], BF16, tag="kT",
                                       name=f"kT{p}")
                pk = ps.tile([128, 512], F32, tag="acc", bufs=2, name="pk")
                for di in range(4):
                    d = 2 * di
                    nc.tensor.matmul(pk, wk_ss[p][:, d:d + 2, :],
                                     xT[:, d:d + 2, 512 * n:512 * (n + 1)],
                                     start=(di == 0), stop=(di == 3),
                                     perf_mode=mybir.MatmulPerfMode.DoubleRow)
                nc.vector.tensor_scalar_add(
                    out=kT_s[p][:, n, :], in0=pk, scalar1=bk_t[:, p:p + 1])

            def emit_v_piece(p, n):
                if n == 0:
                    va_s[p] = kvp.tile([128, NT, 2, 72], FP8, tag="va",
                                       name=f"va{p}")
                    nc.vector.memset(va_s[p][:, :, :, 64:65], 1.0)
                pv = ps.tile([128, 512], F32, tag="acc", bufs=2, name="pv")
                for di in range(4):
                    d = 2 * di
                    nc.tensor.matmul(pv, wv_ss[p][:, d:d + 2, :],
                                     xT[:, d:d + 2, 512 * n:512 * (n + 1)],
                                     start=(di == 0), stop=(di == 3),
                                     perf_mode=mybir.MatmulPerfMode.DoubleRow)
                vts_t = vtsp.tile([128, 512], BF16, tag="vts", name="vts")
                nc.vector.tensor_scalar_add(
                    out=vts_t, in0=pv, scalar1=bv_t[:, p:p + 1])
                # PE transpose V^T -> V, 4 token blocks into one PSUM bank
                tr = ps.tile([128, 4, 128], BF16, tag="acc", bufs=2,
                             name="tr")
                for s in range(4):
                    nc.tensor.transpose(
                        tr[:, s, :], vts_t[:, 128 * s:128 * (s + 1)],
                        ident_s)
                for s in range(4):
                    t = 4 * n + s
                    nc.vector.tensor_copy(
                        out=va_s[p][:, t, :, 0:64],
                        in_=tr[:, s, :].rearrange("p (h l) -> p h l", h=2))

            # pair 0 projections up front (paced by the x DMAs)
            emit_wdma(0)
            # phase-D inputs queued on gpsimd behind the hot weight loads
            lnc = bigp.tile([128, 2, D], F32)
            nc.gpsimd.dma_start(out=lnc[:, 0, :],
                                in_=bcast_rows(gamma_d[None], 128))
            nc.gpsimd.dma_start(out=lnc[:, 1, :],
                                in_=bcast_rows(beta_d[None], 128))
            gamma_b, beta_b = lnc[:, 0, :], lnc[:, 1, :]
            lnc_bf = bigp.tile([128, 2, D], BF16)
            nc.vector.tensor_copy(out=lnc_bf, in_=lnc)
            gamma_bf, beta_bf = lnc_bf[:, 0, :], lnc_bf[:, 1, :]
            xqb_t = bigp.tile([128, NTQ, D], F32)
            nc.gpsimd.dma_start(
                out=xqb_t, in_=xqb_d.rearrange("(i p) d -> p i d", p=128))
            for n in range(4):
                emit_k_piece(0, n)
            for n in range(4):
                emit_v_piece(0, n)

            # =========================================================
            # Attention, software-pipelined across pairs
            # =========================================================
            outT = bigp.tile([128, ND, TQ], FP8)
            wo_t = bigp.tile([128, ND, D], FP8)  # DMA'd as pair-7 work

            def emit_scores_group(p, ch, g, exs):
                # per-head score buffers: the two streams ping-pong on
                # separate PSUM tags so scores(k+1) of head A only waits on
                # ACT_a(k), never on the full slot's worth of exp work
                psc_a = ps.tile([128, 2, 512], F32, tag="sca", name="psc_a")
                psc_b = ps.tile([128, 2, 512], F32, tag="scb", name="psc_b")
                for s2 in range(2):
                    t = ch * 4 + g * 2 + s2
                    nt, tt = t // 4, t % 4
                    ksl = kT_s[p][:, nt, 128 * tt:128 * (tt + 1)]
                    # two heads adjacent, disjoint row groups -> run
                    # concurrently in the PE array
                    nc.tensor.matmul(psc_a[:, s2, :], ksl[0:64, :],
                                     qT[0:64, p, :], start=True, stop=True)
                    nc.tensor.matmul(psc_b[:, s2, :], ksl[64:128, :],
                                     qT[64:128, p, :], start=True, stop=True)
                ex_a = expp.tile([128, 2, 512], FP8, tag="exa", name="ex_a")
                nc.scalar.activation(
                    out=ex_a, in_=psc_a,
                    func=mybir.ActivationFunctionType.Exp, scale=SCALE / 256.0)
                ex_b = expp.tile([128, 2, 512], FP8, tag="exb", name="ex_b")
                nc.scalar.activation(
                    out=ex_b, in_=psc_b,
                    func=mybir.ActivationFunctionType.Exp, scale=SCALE / 256.0)
                exs[(ch, g)] = (ex_a, ex_b)

            def emit_attnv_group(p, ch, g, exs, pav):
                # fp8 DoubleRow: one matmul contracts BOTH key tiles of the
                # group (2 x 128 keys), ~1.44x PE throughput
                ex_pair = exs[(ch, g)]
                t = ch * 4 + g * 2
                for he in range(2):
                    nc.tensor.matmul(
                        pav[he][0:65, :],
                        va_s[p][:, t:t + 2, he, 0:65],
                        ex_pair[he],
                        start=(t == 0), stop=(t == NT - 2),
                        perf_mode=mybir.MatmulPerfMode.DoubleRow)

            def emit_normalize(p, pav):
                # denominator: row 64 of pav -> K=1 matmul broadcast to 64
                # partitions -> reciprocal -> multiply.
                for he in range(2):
                    dns = smallp.tile([128, TQ], BF16, tag="dns", name="dns")
                    nc.vector.tensor_copy(out=dns[64:65, :],
                                          in_=pav[he][64:65, :])
                    rb = ps.tile([128, 512], F32, tag="acc", bufs=2,
                                 name="rb")
                    nc.tensor.matmul(rb[0:64, :], ones_r[64:65, :],
                                     dns[64:65, :], start=True, stop=True)
                    scr = smallp.tile([128, TQ], F32, tag="scr", name="scr")
                    rrec = smallp.tile([128, TQ], F32, tag="rrec",
                                       name="rrec")
                    nc.vector.reciprocal_approx_accurate(
                        out=rrec[0:64, :], in_=rb[0:64, :],
                        scratch=scr[0:64, :])
                    if he == 0:
                        nc.vector.tensor_mul(
                            out=outT[0:64, p, :],
                            in0=pav[he][0:64, :], in1=rrec[0:64, :])
                    else:
                        tmp = smallp.tile([128, TQ], FP8, tag="tmp",
                                          name="tmp")
                        nc.vector.tensor_mul(
                            out=tmp[0:64, :],
                            in0=pav[he][0:64, :], in1=rrec[0:64, :])
                        nc.gpsimd.dma_start(
                            out=outT[64:128, p, :], in_=tmp[0:64, :])

            # Flat slot stream (p, ch, g): scores for slot k, attn.V for
            # slot k-1, one projection piece -- the lag-1 attn.V crosses
            # pair boundaries so the PE always has ~2us of queued work
            # while the exp ACT for the freshly issued scores completes.
            exs = {}
            pavs = {}
            work = []
            prev = None
            norm_pending = None
            slots = [(p, ch, g) for p in range(NPAIR)
                     for ch in range(4) for g in range(2)]
            for (p, ch, g) in slots:
                if ch == 0 and g == 0:
                    pavs[p] = [ps.tile([128, 512], F32, tag="pav", bufs=2,
                                       name=f"pav{p}_{he}")
                               for he in range(2)]
                    if p + 1 < NPAIR:
                        work.append(lambda pp=p + 1: emit_wdma(pp))
                        for n in range(4):
                            work.append(lambda pp=p + 1, nn=n:
                                        emit_k_piece(pp, nn))
                        for n in range(4):
                            work.append(lambda pp=p + 1, nn=n:
                                        emit_v_piece(pp, nn))
                    else:
                        work.append(
                            lambda: nc.gpsimd.dma_start(out=wo_t, in_=wo_d))
                emit_scores_group(p, ch, g, exs)
                if prev is not None:
                    pp, pch, pg = prev
                    emit_attnv_group(pp, pch, pg, exs, pavs[pp])
                    if (pch, pg) == (3, 1):
                        norm_pending = pp
                if work:
                    work.pop(0)()
                if ch == 3 and g == 1 and work:
                    work.pop(0)()  # keep up: 9 pieces arrive per pair
                if norm_pending is not None and (ch, g) == (1, 0):
                    # deferred 2 extra slots: the dns DVE copy has drained
                    # by the time the PE reaches the rb broadcast matmul
                    emit_normalize(norm_pending, pavs.pop(norm_pending))
                    norm_pending = None
                prev = (p, ch, g)
            emit_attnv_group(7, 3, 1, exs, pavs[7])
            emit_normalize(7, pavs.pop(7))
            while work:
                work.pop(0)()

            # =========================================================
            # Out-projection + residual + LayerNorm
            # =========================================================
            for i in range(NTQ):
                # two PSUM halves on different tags -> i and i+1 overlap
                po_h = [ps.tile([128, 512], F32, tag="pav", bufs=2,
                                name="poa"),
                        ps.tile([128, 512], F32, tag="acc", bufs=2,
                                name="pob")]
                for half in range(2):
                    for di in range(4):
                        d = 2 * di
                        nc.tensor.matmul(
                            po_h[half],
                            outT[:, d:d + 2, 128 * i:128 * (i + 1)],
                            wo_t[:, d:d + 2, 512 * half:512 * (half + 1)],
                            start=(di == 0), stop=(di == 3),
                            perf_mode=mybir.MatmulPerfMode.DoubleRow)
                ysb = ybufp.tile([128, D], F32, tag="ysb", name="ysb")
                # y = out + (x + bo)  (bo folded into xqb on host)
                for half in range(2):
                    nc.vector.tensor_add(
                        out=ysb[:, 512 * half:512 * (half + 1)],
                        in0=po_h[half],
                        in1=xqb_t[:, i, 512 * half:512 * (half + 1)])
                # LayerNorm
                stats = smallp.tile([128, 2, 6], F32, tag="stats")
                mv = smallp.tile([128, 2], F32, tag="mv")
                yv = ysb.rearrange("p (a b) -> p a b", a=2)
                for sg in range(2):
                    nc.vector.bn_stats(out=stats[:, sg, :], in_=yv[:, sg, :])
                nc.vector.bn_aggr(out=mv, in_=stats)
                sd = smallp.tile([128, 1], F32, tag="sd")
                nc.scalar.activation(out=sd, in_=mv[:, 1:2],
                                     func=mybir.ActivationFunctionType.Sqrt,
                                     bias=eps_t, scale=1.0)
                rstd = smallp.tile([128, 1], F32, tag="rstd")
                nc.vector.reciprocal(out=rstd, in_=sd)
                # post-stats chain entirely on DVE in bf16 (2x rate); the
                # normalized values are O(1) so bf16 rounding is ~0.4%
                ysb2 = ybufp.tile([128, D], BF16, tag="ysb2", name="ysb2")
                nc.vector.tensor_scalar(
                    out=ysb2, in0=ysb, scalar1=mv[:, 0:1],
                    scalar2=rstd, op0=mybir.AluOpType.subtract,
                    op1=mybir.AluOpType.mult)
                nc.vector.tensor_mul(out=ysb2, in0=ysb2, in1=gamma_bf)
                nc.vector.tensor_add(out=ysb, in0=ysb2, in1=beta_bf)
                nc.sync.dma_start(out=y_d[128 * i:128 * (i + 1), :],
                                  in_=ysb)

    nc.compile()
    return nc


_PROGRAM_CACHE = {}


def _get_program():
    if "p" not in _PROGRAM_CACHE:
        _PROGRAM_CACHE["p"] = _build_program()
    return _PROGRAM_CACHE["p"]


def _pack_w(w):
    # [p, otile, dtile, c] = W[128*dtile+p, 128*otile+c], x16 into fp8's
    # normal range (compensated via exp scale / ones column on device)
    w = np.asarray(w, np.float32).reshape(ND, 128, ND, 128)
    return np.ascontiguousarray(
        w.transpose(1, 2, 0, 3) * 16.0).astype(ml_dtypes.float8_e4m3fn)


def _pack_wo(w):
    # [p, dtile, o] = 16*W[128*dtile+p, o], fp8 (outT carries x16 as well;
    # the combined x256 also scales the residual and cancels in LayerNorm)
    w = np.asarray(w, np.float32).reshape(ND, 128, D)
    return np.ascontiguousarray(
        w.transpose(1, 0, 2) * 16.0).astype(ml_dtypes.float8_e4m3fn)


def _pack_b(b):
    # [p, otile] = 16*b[128*otile+p]  (matches the x16 weight scaling)
    b = np.asarray(b, np.float32).reshape(ND, 128)
    return np.ascontiguousarray(b.transpose(1, 0)) * 16.0


def kernel(x, Wq, bq, Wk, bk, Wv, bv, Wo, bo, gamma, beta, _trace=False):
    x = np.asarray(x, dtype=np.float32)
    nc = _get_program()

    wq_p, wk_p, wv_p = _pack_w(Wq), _pack_w(Wk), _pack_w(Wv)
    wo_p = _pack_wo(Wo)
    bq_p, bk_p, bv_p = _pack_b(bq), _pack_b(bk), _pack_b(bv)
    bo_f = np.asarray(bo, np.float32)
    in_maps = []
    for c in range(N_CORES):
        b = c // CORES_PER_BATCH
        off = TQ * (c % CORES_PER_BATCH)
        xb = np.concatenate([x[b, off:], x[b, :off]], axis=0)
        xbT = np.ascontiguousarray(
            xb.T.reshape(ND, 128, S).transpose(1, 0, 2)).astype(
                ml_dtypes.float8_e4m3fn)
        in_maps.append({
            "xbT": xbT,
            "xqb": (np.ascontiguousarray(xb[0:TQ]) + bo_f) * 256.0,
            "wq": wq_p, "wk": wk_p, "wv": wv_p, "wo": wo_p,
            "bq": bq_p, "bk": bk_p, "bv": bv_p,
            "gamma": np.asarray(gamma, np.float32),
            "beta": np.asarray(beta, np.float32),
        })

    res = bass_utils.run_bass_kernel_spmd(
        nc, in_maps, list(range(N_CORES)), trace=_trace)

    y = np.empty((B, S, D), dtype=np.float32)
    for c in range(N_CORES):
        b = c // CORES_PER_BATCH
        off = TQ * (c % CORES_PER_BATCH)
        y[b, off:off + TQ] = res.results[c]["y"]

    kernel.last_exec_time_ns = res.exec_time_ns
    return y


kernel.last_exec_time_ns = None
